# revision 14
# baseline (speedup 1.0000x reference)
"""Trainium2 Bass kernel for nn_MLP_Route_RL_Model (route RL model).

Reference math (per batch element b of 256):
  - state = [route_nums (48) | customers (48*24*36)]
  - customer MLP (tanh-tanh, 36->128->32) on every node of every route
  - 2-layer GRU (hidden 128) over the 24 nodes of each of the 48 routes
  - route summary mean, node-selection MLP 256->256->128->24, masked softmax

Sharding: pure data parallel over batch B=256 -> 8 cores x 32.

Layout: feature-major activations ([feature, token] in SBUF) so matmuls
contract over the partition dim without transposes.

v4 schedule notes (the kernel is ACT elementwise-bound; engine cost is
free-dim elements only, so fewer/bigger ACT ops win):
  - r|z gate pre-activations share one 2-bank PSUM tile [128,1024]; with
    the (all-zero) biases dropped, ONE sigmoid covers both gates.
  - The n-gate input for all three 512-token chunks accumulates into one
    3-bank PSUM tile [128,1536]: x-side matmul + an identity matmul that
    injects t_ = r*ph (computed on DVE) into the bank; ONE tanh covers
    the full token width. This also deletes the s_ = pi + t_ DVE add.
  - h' = n + z*(h-n): 3 DVE ops/chunk at fp16 2x.
  - PSUM: przb (2 banks x2) + pi3 (3 banks x1) + ph (1) = 8 banks; the
    customer MLP borrows przb slots at low scheduler priority.
  - A non-zero-bias fallback keeps split sigmoids + bias ports.
"""

import os
import sys

import numpy as np

sys.path.insert(0, "/opt/trn_rl_repo")

import concourse.bass as bass  # noqa: E402
import concourse.bacc as bacc  # noqa: E402
import concourse.mybir as mybir  # noqa: E402
import concourse.tile as tile  # noqa: E402
from concourse.bass_utils import run_bass_kernel_spmd  # noqa: E402

F32 = mybir.dt.float32
F16 = mybir.dt.float16
AF = mybir.ActivationFunctionType
OP = mybir.AluOpType

# Problem shape constants
B = 256
NCORES = 8
BLOC = B // NCORES          # 32 batch rows per core
MR = 48                     # routes per batch
MN = 24                     # nodes per route
FEAT = 36
CH = 128                    # customer hidden
CO = 32                     # customer out
GH = 128                    # GRU hidden
S = BLOC * MR               # sequences per core = 1536
NC = 512                    # token chunk (PSUM bank width in fp32)
NCH = S // NC               # chunks per core = 3
NG = MN // 4                # node groups of 4 (cust_out partition stacking)

_cache = {}


def _build(reps=1, zb=True):
    """Trace + schedule the per-core Tile kernel. zb: all biases are zero."""
    nc = bacc.Bacc("TRN2", target_bir_lowering=False, debug=False)

    # ---- DRAM I/O ----------------------------------------------------------
    d_cust = nc.dram_tensor("cust_fm", [FEAT, MN * S], F16, kind="ExternalInput")
    d_rn = nc.dram_tensor("rn_pm", [128, S // 128], F32, kind="ExternalInput")
    d_wc1 = nc.dram_tensor("Wc1h", [FEAT, CH], F16, kind="ExternalInput")
    d_wc2 = nc.dram_tensor("Wc2h", [CH, CO], F16, kind="ExternalInput")
    d_wih0 = nc.dram_tensor("Wih0h", [128, 3 * GH], F16, kind="ExternalInput")
    d_whh0 = nc.dram_tensor("Whh0h", [GH, 3 * GH], F16, kind="ExternalInput")
    d_wih1 = nc.dram_tensor("Wih1h", [GH, 3 * GH], F16, kind="ExternalInput")
    d_whh1 = nc.dram_tensor("Whh1h", [GH, 3 * GH], F16, kind="ExternalInput")
    d_ident = nc.dram_tensor("ident128", [128, 128], F16, kind="ExternalInput")
    if not zb:
        d_bc1 = nc.dram_tensor("bc1", [CH, 1], F32, kind="ExternalInput")
        d_bc2 = nc.dram_tensor("bc2s", [128, 1], F32, kind="ExternalInput")
        d_gb = {}
        for layer in (0, 1):
            for g in ("r", "z", "in", "hn"):
                d_gb[(layer, g)] = nc.dram_tensor(
                    f"b{layer}_{g}", [GH, 1], F32, kind="ExternalInput"
                )
        d_bn1 = nc.dram_tensor("bn1c", [128, 2], F32, kind="ExternalInput")
        d_bn2 = nc.dram_tensor("bn2c", [128, 1], F32, kind="ExternalInput")
        d_bn3 = nc.dram_tensor("bn3r", [1, MN], F32, kind="ExternalInput")
        d_ones = nc.dram_tensor("ones128", [1, 128], F32, kind="ExternalInput")
    d_wn1a = nc.dram_tensor("Wn1a", [GH, 256], F16, kind="ExternalInput")
    d_wn1b = nc.dram_tensor("Wn1b", [GH, 256], F16, kind="ExternalInput")
    d_wn2a = nc.dram_tensor("Wn2a", [128, 128], F16, kind="ExternalInput")
    d_wn2b = nc.dram_tensor("Wn2b", [128, 128], F16, kind="ExternalInput")
    d_wn3 = nc.dram_tensor("Wn3h", [GH, MN], F16, kind="ExternalInput")
    d_sel = nc.dram_tensor("sel", [BLOC, S], F16, kind="ExternalInput")
    d_iota = nc.dram_tensor("iota24", [128, MN], F32, kind="ExternalInput")
    d_out = nc.dram_tensor("out_tm", [S, MN], F32, kind="ExternalOutput")

    with tile.TileContext(nc) as tc:
        with (
            tc.tile_pool(name="wpool", bufs=1) as wp,
            tc.tile_pool(name="state", bufs=1) as sp,
            tc.tile_pool(name="xin", bufs=10) as xp,
            tc.tile_pool(name="h1c", bufs=6) as h1p,
            tc.tile_pool(name="wk", bufs=6) as wk,
            tc.tile_pool(name="nfw", bufs=3) as nfw,
            tc.tile_pool(name="fin", bufs=4) as fp_,
            tc.tile_pool(name="ps2", bufs=2, space="PSUM") as ps2,
            tc.tile_pool(name="ps3", bufs=1, space="PSUM") as ps3,
            tc.tile_pool(name="ps1", bufs=1, space="PSUM") as ps1,
        ):
            def lowprio():
                # deprioritize: scheduler runs these only in recurrence gaps
                return tc.high_priority(offset=-1_000_000)

            # ---- load weights / constants (A-critical ones first) ----------
            def wtile(dram, shape, dtype):
                t = wp.tile(shape, dtype, tag=dram.name)
                nc.sync.dma_start(t[:], dram.ap())
                return t

            wc1 = wtile(d_wc1, [FEAT, CH], F16)
            wc2 = wtile(d_wc2, [CH, CO], F16)
            ident = wtile(d_ident, [128, 128], F16)
            wih0 = wtile(d_wih0, [128, 3 * GH], F16)
            whh0 = wtile(d_whh0, [GH, 3 * GH], F16)
            wih1 = wtile(d_wih1, [128, 3 * GH], F16)
            whh1 = wtile(d_whh1, [GH, 3 * GH], F16)
            gb = {}
            if not zb:
                bc1 = wtile(d_bc1, [CH, 1], F32)
                bc2 = wtile(d_bc2, [128, 1], F32)
                for k, d in d_gb.items():
                    gb[k] = wtile(d, [GH, 1], F32)
            with lowprio():
                wn1a = wtile(d_wn1a, [GH, 256], F16)
                wn1b = wtile(d_wn1b, [GH, 256], F16)
                wn2a = wtile(d_wn2a, [128, 128], F16)
                wn2b = wtile(d_wn2b, [128, 128], F16)
                wn3 = wtile(d_wn3, [GH, MN], F16)
                sel = wtile(d_sel, [BLOC, S], F16)
                iota24 = wtile(d_iota, [128, MN], F32)
                rn_pm = wtile(d_rn, [128, S // 128], F32)
                if not zb:
                    bn1 = wtile(d_bn1, [128, 2], F32)
                    bn2 = wtile(d_bn2, [128, 1], F32)
                    bn3 = wtile(d_bn3, [1, MN], F32)
                    ones128 = wtile(d_ones, [1, 128], F32)

            # persistent state: customer-MLP output, GRU hidden states
            # cust_out layout: partition = (n%4)*32 + f, free = (n//4)*S + s
            cust = sp.tile([128, NG * S], F16, tag="cust_out")
            h1 = sp.tile([GH, S], F16, tag="h1")
            h2 = sp.tile([GH, S], F16, tag="h2")

          # timing-calibration repeat loop (reps=1 in production)
          # fmt: off
            for _rep in range(reps):
              nc.vector.memset(h1[:], 0.0)
              nc.gpsimd.memset(h2[:], 0.0)

              # ---- phase A: customer MLP (gap filler) ----------------------
              # p1 pre-activations for node pairs share a przb 2-bank tile;
              # one tanh covers both (bc1 is per-partition so this also
              # holds in the non-zb fallback).
              xtiles = {}
              import contextlib
              def emitA(g, lowp):
                with (lowprio() if lowp else contextlib.nullcontext()):
                  for sb in range(NCH):
                      # stage 1: h1c for the 4 nodes (2 przb borrows, one at
                      # a time); stage 2: c2 accumulation (1 przb borrow).
                      h1cbs = []
                      for kp in range(2):
                          p1b = ps2.tile([CH, 2 * NC], F32, tag="przb",
                                         name=f"p1b_{g}_{sb}_{kp}")
                          h1cb = h1p.tile([CH, 2 * NC], F16, tag="h1c")
                          for kk in range(2):
                              k = 2 * kp + kk
                              n = 4 * g + k
                              if n not in xtiles:
                                  xn = xp.tile([FEAT, S], F16, tag="xc", name=f"xc{n}")
                                  nc.sync.dma_start(
                                      xn[:], d_cust.ap()[:, n * S : (n + 1) * S]
                                  )
                                  xtiles[n] = xn
                              xc = xtiles[n]
                              nc.tensor.matmul(
                                  p1b[:, kk * NC : (kk + 1) * NC], wc1[:],
                                  xc[:, sb * NC : (sb + 1) * NC],
                              )
                          if zb:
                              nc.scalar.activation(h1cb[:], p1b[:], AF.Tanh)
                          else:
                              nc.scalar.activation(h1cb[:], p1b[:], AF.Tanh, bias=bc1[:])
                          h1cbs.append(h1cb)
                      c2 = ps2.tile([128, NC], F32, tag="przb", name=f"c2_{g}_{sb}")
                      for k in range(4):
                          nc.tensor.matmul(
                              c2[32 * k : 32 * (k + 1), :], wc2[:],
                              h1cbs[k // 2][:, (k % 2) * NC : (k % 2 + 1) * NC],
                              tile_position=(0, 32 * k),
                          )
                      if zb:
                          nc.scalar.activation(
                              cust[:, g * S + sb * NC : g * S + (sb + 1) * NC],
                              c2[:], AF.Tanh,
                          )
                      else:
                          nc.scalar.activation(
                              cust[:, g * S + sb * NC : g * S + (sb + 1) * NC],
                              c2[:], AF.Tanh, bias=bc2[:],
                          )

              # ---- phase B: 2-layer GRU over MN steps -----------------------
              def emitB_layer(t, layer, h, wih, whh, kq):
                  """One GRU layer update for step t on hidden h [GH, S]."""
                  g = t // 4
                  pi3 = ps3.tile([GH, S], F32, tag="pi3")
                  n3 = nfw.tile([GH, S], F16, tag="n3")
                  rzs = []
                  for c in range(NCH):
                      c0, c1 = c * NC, (c + 1) * NC
                      hc = h[:, c0:c1]
                      przb = ps2.tile([GH, 2 * NC], F32, tag="przb")
                      pr = przb[:, 0:NC]
                      pz = przb[:, NC : 2 * NC]
                      ph = ps1.tile([GH, NC], F32, tag="ph")
                      pic = pi3[:, c0:c1]
                      if kq is not None:
                          p0 = 32 * kq
                          tp = (p0, 0)
                          xc = cust[p0 : p0 + CO, g * S + c0 : g * S + c1]
                          wx = wih[p0 : p0 + CO, :]
                          nc.tensor.matmul(pr, whh[:, 0:GH], hc, start=True, stop=False)
                          nc.tensor.matmul(pr, wx[:, 0:GH], xc, start=False, stop=True,
                                           tile_position=tp)
                          nc.tensor.matmul(pz, whh[:, GH : 2 * GH], hc, start=True, stop=False)
                          nc.tensor.matmul(pz, wx[:, GH : 2 * GH], xc, start=False, stop=True,
                                           tile_position=tp)
                          nc.tensor.matmul(ph[:], whh[:, 2 * GH : 3 * GH], hc)
                          nc.tensor.matmul(pic, wx[:, 2 * GH : 3 * GH], xc,
                                           start=True, stop=False, tile_position=tp)
                      else:
                          xc = h1[:, c0:c1]
                          nc.tensor.matmul(pr, whh[:, 0:GH], hc, start=True, stop=False)
                          nc.tensor.matmul(pr, wih[:, 0:GH], xc, start=False, stop=True)
                          nc.tensor.matmul(pz, whh[:, GH : 2 * GH], hc, start=True, stop=False)
                          nc.tensor.matmul(pz, wih[:, GH : 2 * GH], xc, start=False, stop=True)
                          nc.tensor.matmul(ph[:], whh[:, 2 * GH : 3 * GH], hc)
                          nc.tensor.matmul(pic, wih[:, 2 * GH : 3 * GH], xc,
                                           start=True, stop=False)
                      rz = wk.tile([GH, 2 * NC], F16, tag="rz")
                      if zb:
                          nc.scalar.activation(rz[:], przb[:], AF.Sigmoid)
                      else:
                          nc.scalar.activation(rz[:, 0:NC], pr, AF.Sigmoid,
                                               bias=gb[(layer, "r")][:])
                          nc.scalar.activation(rz[:, NC : 2 * NC], pz, AF.Sigmoid,
                                               bias=gb[(layer, "z")][:])
                      t_c = wk.tile([GH, NC], F16, tag="t_")
                      if zb:
                          nc.vector.tensor_mul(t_c[:], ph[:], rz[:, 0:NC])
                      else:
                          nc.vector.scalar_tensor_tensor(
                              t_c[:], ph[:], gb[(layer, "hn")][:], rz[:, 0:NC],
                              OP.add, OP.mult,
                          )
                      # inject t_ into the pi bank: pi += I @ t_
                      nc.tensor.matmul(pic, ident[:], t_c[:], start=False, stop=True)
                      rzs.append(rz)
                  # n = tanh(pi3) over the full 1536-token width (one ACT op)
                  if zb:
                      nc.scalar.activation(n3[:], pi3[:], AF.Tanh)
                  else:
                      nc.scalar.activation(n3[:], pi3[:], AF.Tanh,
                                           bias=gb[(layer, "in")][:])
                  # h' = n + z*(h - n), per chunk, all DVE at fp16 2x
                  for c in range(NCH):
                      c0, c1 = c * NC, (c + 1) * NC
                      d_c = wk.tile([GH, NC], F16, tag="d_")
                      nc.vector.tensor_sub(d_c[:], h[:, c0:c1], n3[:, c0:c1])
                      e_c = wk.tile([GH, NC], F16, tag="e_")
                      nc.vector.tensor_mul(e_c[:], rzs[c][:, NC : 2 * NC], d_c[:])
                      nc.vector.tensor_add(h[:, c0:c1], n3[:, c0:c1], e_c[:])

              for t in range(MN):
                  if t % 4 == 0:
                      emitA(t // 4, lowp=(t > 0))
                  emitB_layer(t, 0, h1, wih0, whh0, t % 4)
                  emitB_layer(t, 1, h2, wih1, whh1, None)

              # ---- phase C: route mean + node MLP + masked softmax ----------
              mean32 = fp_.tile([GH, BLOC], F32, tag="mean32")
              h2v = h2[:].rearrange("p (b r) -> p b r", r=MR)
              nc.vector.tensor_reduce(mean32[:], h2v, mybir.AxisListType.X, OP.add)
              mean = fp_.tile([GH, BLOC], F16, tag="mean")
              nc.vector.tensor_copy(mean[:], mean32[:])
              pmt = ps1.tile([BLOC, 256], F32, tag="ph")
              nc.tensor.matmul(pmt[:], mean[:], wn1b[:])
              mmt = fp_.tile([BLOC, 256], F16, tag="mmt")
              nc.vector.tensor_copy(mmt[:], pmt[:])

              for c in range(NCH):
                  c0, c1 = c * NC, (c + 1) * NC
                  n1 = []
                  for m in range(2):
                      p1 = ps2.tile([128, NC], F32, tag="przb", name=f"cp1_{c}_{m}")
                      nc.tensor.matmul(
                          p1[:], wn1a[:, 128 * m : 128 * (m + 1)], h2[:, c0:c1],
                          start=True, stop=False,
                      )
                      nc.tensor.matmul(
                          p1[:], mmt[:, 128 * m : 128 * (m + 1)], sel[:, c0:c1],
                          start=False, stop=True,
                      )
                      a1 = fp_.tile([128, NC], F16, tag=f"n1_{m}")
                      if zb:
                          nc.scalar.activation(a1[:], p1[:], AF.Relu)
                      else:
                          nc.scalar.activation(a1[:], p1[:], AF.Relu,
                                               bias=bn1[:, m : m + 1])
                      n1.append(a1)
                  p2 = ps1.tile([128, NC], F32, tag="ph", name=f"cp2_{c}")
                  nc.tensor.matmul(p2[:], wn2a[:], n1[0][:], start=True, stop=False)
                  nc.tensor.matmul(p2[:], wn2b[:], n1[1][:], start=False, stop=True)
                  n2 = fp_.tile([128, NC], F16, tag="n2")
                  if zb:
                      nc.scalar.activation(n2[:], p2[:], AF.Relu)
                  else:
                      nc.scalar.activation(n2[:], p2[:], AF.Relu, bias=bn2[:])
                  for q in range(NC // 128):
                      tok0 = c0 + q * 128
                      pl = ps3.tile([128, MN], F32, tag="pi3", name=f"cpl_{c}_{q}")
                      if zb:
                          nc.tensor.matmul(pl[:], n2[:, q * 128 : (q + 1) * 128], wn3[:])
                      else:
                          nc.tensor.matmul(
                              pl[:], n2[:, q * 128 : (q + 1) * 128], wn3[:],
                              start=True, stop=False,
                          )
                          nc.tensor.matmul(pl[:], ones128[:], bn3[:],
                                           start=False, stop=True)
                      ex = fp_.tile([128, MN], F32, tag="ex")
                      sm = fp_.tile([128, 1], F32, tag="sm")
                      nc.scalar.activation(ex[:], pl[:], AF.Exp, accum_out=sm[:])
                      rec = fp_.tile([128, 1], F32, tag="rec")
                      nc.vector.reciprocal(rec[:], sm[:])
                      msk = fp_.tile([128, MN], F32, tag="msk")
                      nc.vector.tensor_scalar(
                          msk[:], iota24[:], rn_pm[:, tok0 // 128 : tok0 // 128 + 1],
                          None, OP.is_lt
                      )
                      po = fp_.tile([128, MN], F32, tag="po")
                      nc.vector.scalar_tensor_tensor(
                          po[:], ex[:], rec[:], msk[:], OP.mult, OP.mult
                      )
                      nc.sync.dma_start(d_out.ap()[tok0 : tok0 + 128, :], po[:])

    nc.compile()
    return nc


def _prep_inputs(inputs, zb):
    """Host-side preprocessing -> list of per-core input dicts."""
    state = np.ascontiguousarray(inputs["state"], dtype=np.float32)
    rn = state[:, :MR]                                    # [B, 48]
    cust = state[:, MR:].reshape(B, MR, MN, FEAT)

    def f32(x):
        return np.ascontiguousarray(np.asarray(x, dtype=np.float32))

    Wih0 = f32(inputs["Wih0"]); Whh0 = f32(inputs["Whh0"])
    Wih1 = f32(inputs["Wih1"]); Whh1 = f32(inputs["Whh1"])

    com = {
        "Wc1h": np.ascontiguousarray(np.asarray(inputs["Wc1"], np.float16)),
        "Wc2h": np.ascontiguousarray(np.asarray(inputs["Wc2"], np.float16)),
        "Wih0h": np.ascontiguousarray(np.tile(np.asarray(Wih0, np.float16), (4, 1))),
        "Whh0h": Whh0.astype(np.float16),
        "Wih1h": Wih1.astype(np.float16),
        "Whh1h": Whh1.astype(np.float16),
        "Wn1a": f32(inputs["Wn1"])[0:GH, :].astype(np.float16),
        "Wn1b": (f32(inputs["Wn1"])[GH:, :] / np.float32(MR)).astype(np.float16),
        "Wn2a": f32(inputs["Wn2"])[0:128, :].astype(np.float16),
        "Wn2b": f32(inputs["Wn2"])[128:256, :].astype(np.float16),
        "Wn3h": np.asarray(inputs["Wn3"], np.float16),
        "iota24": np.tile(np.arange(MN, dtype=np.float32), (128, 1)),
        "ident128": np.eye(128, dtype=np.float16),
    }
    if not zb:
        bih0 = f32(inputs["bih0"]); bhh0 = f32(inputs["bhh0"])
        bih1 = f32(inputs["bih1"]); bhh1 = f32(inputs["bhh1"])
        com.update({
            "bc1": f32(inputs["bc1"]).reshape(CH, 1),
            "bc2s": np.tile(f32(inputs["bc2"]).reshape(CO), 4).reshape(128, 1),
            "b0_r": (bih0[0:GH] + bhh0[0:GH]).reshape(GH, 1),
            "b0_z": (bih0[GH : 2 * GH] + bhh0[GH : 2 * GH]).reshape(GH, 1),
            "b0_in": bih0[2 * GH :].reshape(GH, 1),
            "b0_hn": bhh0[2 * GH :].reshape(GH, 1),
            "b1_r": (bih1[0:GH] + bhh1[0:GH]).reshape(GH, 1),
            "b1_z": (bih1[GH : 2 * GH] + bhh1[GH : 2 * GH]).reshape(GH, 1),
            "b1_in": bih1[2 * GH :].reshape(GH, 1),
            "b1_hn": bhh1[2 * GH :].reshape(GH, 1),
            "bn1c": np.ascontiguousarray(f32(inputs["bn1"]).reshape(2, 128).T),
            "bn2c": f32(inputs["bn2"]).reshape(128, 1),
            "bn3r": f32(inputs["bn3"]).reshape(1, MN),
            "ones128": np.ones((1, 128), np.float32),
        })
    sel = np.zeros((BLOC, S), np.float32)
    sel[np.arange(S) // MR, np.arange(S)] = 1.0
    com["sel"] = sel.astype(np.float16)

    in_maps = []
    for core in range(NCORES):
        b0, b1 = core * BLOC, (core + 1) * BLOC
        # cust_fm[f, n*S + (b*MR+r)] = cust[b, r, n, f]
        cfm = cust[b0:b1].transpose(3, 2, 0, 1).reshape(FEAT, MN * S)
        m = dict(com)
        m["cust_fm"] = np.ascontiguousarray(cfm.astype(np.float16))
        # rn_pm[p, q] = route_num of token q*128+p
        m["rn_pm"] = np.ascontiguousarray(
            rn[b0:b1].reshape(S).reshape(S // 128, 128).T
        )
        in_maps.append(m)
    return in_maps


def _zb(inputs):
    return all(
        float(np.abs(np.asarray(inputs[k], np.float32)).max()) == 0.0
        for k in ("bc1", "bc2", "bih0", "bhh0", "bih1", "bhh1",
                  "bn1", "bn2", "bn3")
    )


def _run(inputs, **kw):
    zb = _zb(inputs)
    key = ("nc", zb)
    if key not in _cache:
        _cache[key] = _build(zb=zb)
    nc = _cache[key]
    _cache["nc"] = nc  # for test harness introspection
    in_maps = _prep_inputs(inputs, zb)
    return run_bass_kernel_spmd(nc, in_maps, core_ids=list(range(NCORES)), **kw)


def kernel(**inputs) -> np.ndarray:
    res = _run(inputs)
    outs = [r["out_tm"] for r in res.results]
    return np.concatenate(outs, axis=0).reshape(B, MR, MN)


# revision 18
# speedup vs baseline: 1.2513x; 1.2513x over previous
"""Trainium2 Bass kernel for nn_MLP_Route_RL_Model (route RL model).

Reference math (per batch element b of 256):
  - state = [route_nums (48) | customers (48*24*36)]
  - customer MLP (tanh-tanh, 36->128->32) on every node of every route
  - 2-layer GRU (hidden 128) over the 24 nodes of each of the 48 routes
  - route summary mean, node-selection MLP 256->256->128->24, masked softmax

Sharding: pure data parallel over batch B=256 -> 8 cores x 32.

Layout: feature-major activations ([feature, token] in SBUF) so matmuls
contract over the partition dim without transposes.

v4 schedule notes (the kernel is ACT elementwise-bound; engine cost is
free-dim elements only, so fewer/bigger ACT ops win):
  - r|z gate pre-activations share one 2-bank PSUM tile [128,1024]; with
    the (all-zero) biases dropped, ONE sigmoid covers both gates.
  - The n-gate input for all three 512-token chunks accumulates into one
    3-bank PSUM tile [128,1536]: x-side matmul + an identity matmul that
    injects t_ = r*ph (computed on DVE) into the bank; ONE tanh covers
    the full token width. This also deletes the s_ = pi + t_ DVE add.
  - h' = n + z*(h-n): 3 DVE ops/chunk at fp16 2x.
  - PSUM: przb (2 banks x2) + pi3 (3 banks x1) + ph (1) = 8 banks; the
    customer MLP borrows przb slots at low scheduler priority.
  - A non-zero-bias fallback keeps split sigmoids + bias ports.
"""

import os
import sys

import numpy as np

sys.path.insert(0, "/opt/trn_rl_repo")

import concourse.bass as bass  # noqa: E402
import concourse.bacc as bacc  # noqa: E402
import concourse.mybir as mybir  # noqa: E402
import concourse.tile as tile  # noqa: E402
from concourse.bass_utils import run_bass_kernel_spmd  # noqa: E402

F32 = mybir.dt.float32
F16 = mybir.dt.float16
AF = mybir.ActivationFunctionType
OP = mybir.AluOpType

# Problem shape constants
B = 256
NCORES = 8
BLOC = B // NCORES          # 32 batch rows per core
MR = 48                     # routes per batch
MN = 24                     # nodes per route
FEAT = 36
CH = 128                    # customer hidden
CO = 32                     # customer out
GH = 128                    # GRU hidden
S = BLOC * MR               # sequences per core = 1536
NC = 512                    # token chunk (PSUM bank width in fp32)
NCH = S // NC               # chunks per core = 3
NG = MN // 4                # node groups of 4 (cust_out partition stacking)

_cache = {}


def _build(reps=1, zb=True):
    """Trace + schedule the per-core Tile kernel. zb: all biases are zero."""
    nc = bacc.Bacc("TRN2", target_bir_lowering=False, debug=False)

    # ---- DRAM I/O ----------------------------------------------------------
    d_cust = nc.dram_tensor("cust_fm", [FEAT, MN * S], F16, kind="ExternalInput")
    d_rn = nc.dram_tensor("rn_pm", [128, S // 128], F32, kind="ExternalInput")
    d_wc1 = nc.dram_tensor("Wc1h", [FEAT, CH], F16, kind="ExternalInput")
    d_wc2 = nc.dram_tensor("Wc2h", [CH, CO], F16, kind="ExternalInput")
    d_wih0 = nc.dram_tensor("Wih0h", [128, 3 * GH], F16, kind="ExternalInput")
    d_whh0 = nc.dram_tensor("Whh0h", [GH, 3 * GH], F16, kind="ExternalInput")
    d_wih1 = nc.dram_tensor("Wih1h", [GH, 3 * GH], F16, kind="ExternalInput")
    d_whh1 = nc.dram_tensor("Whh1h", [GH, 3 * GH], F16, kind="ExternalInput")
    d_ident = nc.dram_tensor("ident128", [128, 128], F16, kind="ExternalInput")
    if not zb:
        d_bc1 = nc.dram_tensor("bc1", [CH, 1], F32, kind="ExternalInput")
        d_bc2 = nc.dram_tensor("bc2s", [128, 1], F32, kind="ExternalInput")
        d_gb = {}
        for layer in (0, 1):
            for g in ("r", "z", "in", "hn"):
                d_gb[(layer, g)] = nc.dram_tensor(
                    f"b{layer}_{g}", [GH, 1], F32, kind="ExternalInput"
                )
        d_bn1 = nc.dram_tensor("bn1c", [128, 2], F32, kind="ExternalInput")
        d_bn2 = nc.dram_tensor("bn2c", [128, 1], F32, kind="ExternalInput")
        d_bn3 = nc.dram_tensor("bn3r", [1, MN], F32, kind="ExternalInput")
        d_ones = nc.dram_tensor("ones128", [1, 128], F32, kind="ExternalInput")
    d_wn1a = nc.dram_tensor("Wn1a", [GH, 256], F16, kind="ExternalInput")
    d_wn1b = nc.dram_tensor("Wn1b", [GH, 256], F16, kind="ExternalInput")
    d_wn2a = nc.dram_tensor("Wn2a", [128, 128], F16, kind="ExternalInput")
    d_wn2b = nc.dram_tensor("Wn2b", [128, 128], F16, kind="ExternalInput")
    d_wn3 = nc.dram_tensor("Wn3h", [GH, MN], F16, kind="ExternalInput")
    d_sel = nc.dram_tensor("sel", [BLOC, S], F16, kind="ExternalInput")
    d_iota = nc.dram_tensor("iota24", [128, MN], F32, kind="ExternalInput")
    d_out = nc.dram_tensor("out_tm", [S, MN], F32, kind="ExternalOutput")

    with tile.TileContext(nc) as tc:
        with (
            tc.tile_pool(name="wpool", bufs=1) as wp,
            tc.tile_pool(name="state", bufs=1) as sp,
            tc.tile_pool(name="xin", bufs=10) as xp,
            tc.tile_pool(name="h1c", bufs=6) as h1p,
            tc.tile_pool(name="wk", bufs=6) as wk,
            tc.tile_pool(name="nfw", bufs=3) as nfw,
            tc.tile_pool(name="fin", bufs=4) as fp_,
            tc.tile_pool(name="ps2", bufs=2, space="PSUM") as ps2,
            tc.tile_pool(name="ps1", bufs=2, space="PSUM") as ps1,
        ):
            def lowprio():
                # deprioritize: scheduler runs these only in recurrence gaps
                return tc.high_priority(offset=-1_000_000)

            # ---- load weights / constants (A-critical ones first) ----------
            def wtile(dram, shape, dtype):
                t = wp.tile(shape, dtype, tag=dram.name)
                nc.sync.dma_start(t[:], dram.ap())
                return t

            wc1 = wtile(d_wc1, [FEAT, CH], F16)
            wc2 = wtile(d_wc2, [CH, CO], F16)
            ident = wtile(d_ident, [128, 128], F16)
            wih0 = wtile(d_wih0, [128, 3 * GH], F16)
            whh0 = wtile(d_whh0, [GH, 3 * GH], F16)
            wih1 = wtile(d_wih1, [128, 3 * GH], F16)
            whh1 = wtile(d_whh1, [GH, 3 * GH], F16)
            gb = {}
            if not zb:
                bc1 = wtile(d_bc1, [CH, 1], F32)
                bc2 = wtile(d_bc2, [128, 1], F32)
                for k, d in d_gb.items():
                    gb[k] = wtile(d, [GH, 1], F32)
            with lowprio():
                wn1a = wtile(d_wn1a, [GH, 256], F16)
                wn1b = wtile(d_wn1b, [GH, 256], F16)
                wn2a = wtile(d_wn2a, [128, 128], F16)
                wn2b = wtile(d_wn2b, [128, 128], F16)
                wn3 = wtile(d_wn3, [GH, MN], F16)
                sel = wtile(d_sel, [BLOC, S], F16)
                iota24 = wtile(d_iota, [128, MN], F32)
                rn_pm = wtile(d_rn, [128, S // 128], F32)
                if not zb:
                    bn1 = wtile(d_bn1, [128, 2], F32)
                    bn2 = wtile(d_bn2, [128, 1], F32)
                    bn3 = wtile(d_bn3, [1, MN], F32)
                    ones128 = wtile(d_ones, [1, 128], F32)

            # persistent state: customer-MLP output, GRU hidden states
            # cust_out layout: partition = (n%4)*32 + f, free = (n//4)*S + s
            cust = sp.tile([128, NG * S], F16, tag="cust_out")
            h1 = sp.tile([GH, S], F16, tag="h1")
            h2 = sp.tile([GH, S], F16, tag="h2")

          # timing-calibration repeat loop (reps=1 in production)
          # fmt: off
            for _rep in range(reps):
              nc.vector.memset(h1[:], 0.0)
              nc.gpsimd.memset(h2[:], 0.0)

              # ---- phase A: customer MLP (gap filler) ----------------------
              # p1 pre-activations for node pairs share a przb 2-bank tile;
              # one tanh covers both (bc1 is per-partition so this also
              # holds in the non-zb fallback).
              xtiles = {}
              import contextlib
              def emitA(g, lowp):
                with (lowprio() if lowp else contextlib.nullcontext()):
                  for sb in range(NCH):
                      # stage 1: h1c for the 4 nodes (2 przb borrows, one at
                      # a time); stage 2: c2 accumulation (1 przb borrow).
                      h1cbs = []
                      for kp in range(2):
                          p1b = ps2.tile([CH, 2 * NC], F32, tag="przb",
                                         name=f"p1b_{g}_{sb}_{kp}")
                          h1cb = h1p.tile([CH, 2 * NC], F16, tag="h1c")
                          for kk in range(2):
                              k = 2 * kp + kk
                              n = 4 * g + k
                              if n not in xtiles:
                                  xn = xp.tile([FEAT, S], F16, tag="xc", name=f"xc{n}")
                                  nc.sync.dma_start(
                                      xn[:], d_cust.ap()[:, n * S : (n + 1) * S]
                                  )
                                  xtiles[n] = xn
                              xc = xtiles[n]
                              nc.tensor.matmul(
                                  p1b[:, kk * NC : (kk + 1) * NC], wc1[:],
                                  xc[:, sb * NC : (sb + 1) * NC],
                              )
                          if zb:
                              nc.scalar.activation(h1cb[:], p1b[:], AF.Tanh)
                          else:
                              nc.scalar.activation(h1cb[:], p1b[:], AF.Tanh, bias=bc1[:])
                          h1cbs.append(h1cb)
                      c2 = ps2.tile([128, NC], F32, tag="przb", name=f"c2_{g}_{sb}")
                      for k in range(4):
                          nc.tensor.matmul(
                              c2[32 * k : 32 * (k + 1), :], wc2[:],
                              h1cbs[k // 2][:, (k % 2) * NC : (k % 2 + 1) * NC],
                              tile_position=(0, 32 * k),
                          )
                      if zb:
                          nc.scalar.activation(
                              cust[:, g * S + sb * NC : g * S + (sb + 1) * NC],
                              c2[:], AF.Tanh,
                          )
                      else:
                          nc.scalar.activation(
                              cust[:, g * S + sb * NC : g * S + (sb + 1) * NC],
                              c2[:], AF.Tanh, bias=bc2[:],
                          )

              # ---- phase B: 2-layer GRU over MN steps -----------------------
              def emitB_layer(t, layer, h, wih, whh, kq):
                  """One GRU layer update for step t on hidden h [GH, S]."""
                  g = t // 4
                  for c in range(NCH):
                      c0, c1 = c * NC, (c + 1) * NC
                      hc = h[:, c0:c1]
                      przb = ps2.tile([GH, 2 * NC], F32, tag="przb")
                      pr = przb[:, 0:NC]
                      pz = przb[:, NC : 2 * NC]
                      ph = ps1.tile([GH, NC], F32, tag="ph")
                      pi = ps1.tile([GH, NC], F32, tag="pi")
                      if kq is not None:
                          p0 = 32 * kq
                          tp = (p0, 0)
                          xc = cust[p0 : p0 + CO, g * S + c0 : g * S + c1]
                          wx = wih[p0 : p0 + CO, :]
                          nc.tensor.matmul(pr, whh[:, 0:GH], hc, start=True, stop=False)
                          nc.tensor.matmul(pr, wx[:, 0:GH], xc, start=False, stop=True,
                                           tile_position=tp)
                          nc.tensor.matmul(pz, whh[:, GH : 2 * GH], hc, start=True, stop=False)
                          nc.tensor.matmul(pz, wx[:, GH : 2 * GH], xc, start=False, stop=True,
                                           tile_position=tp)
                          nc.tensor.matmul(ph[:], whh[:, 2 * GH : 3 * GH], hc)
                      else:
                          xc = h1[:, c0:c1]
                          nc.tensor.matmul(pr, whh[:, 0:GH], hc, start=True, stop=False)
                          nc.tensor.matmul(pr, wih[:, 0:GH], xc, start=False, stop=True)
                          nc.tensor.matmul(pz, whh[:, GH : 2 * GH], hc, start=True, stop=False)
                          nc.tensor.matmul(pz, wih[:, GH : 2 * GH], xc, start=False, stop=True)
                          nc.tensor.matmul(ph[:], whh[:, 2 * GH : 3 * GH], hc)
                      rz = wk.tile([GH, 2 * NC], F16, tag="rz")
                      if zb:
                          nc.scalar.activation(rz[:], przb[:], AF.Sigmoid)
                      else:
                          nc.scalar.activation(rz[:, 0:NC], pr, AF.Sigmoid,
                                               bias=gb[(layer, "r")][:])
                          nc.scalar.activation(rz[:, NC : 2 * NC], pz, AF.Sigmoid,
                                               bias=gb[(layer, "z")][:])
                      t_c = wk.tile([GH, NC], F16, tag="t_")
                      if zb:
                          nc.vector.tensor_mul(t_c[:], ph[:], rz[:, 0:NC])
                      else:
                          nc.vector.scalar_tensor_tensor(
                              t_c[:], ph[:], gb[(layer, "hn")][:], rz[:, 0:NC],
                              OP.add, OP.mult,
                          )
                      # finish the n-gate input inside PSUM: pi = I@t_ + Wih_n@x
                      nc.tensor.matmul(pi[:], ident[:], t_c[:], start=True, stop=False)
                      if kq is not None:
                          nc.tensor.matmul(pi[:], wx[:, 2 * GH : 3 * GH], xc,
                                           start=False, stop=True, tile_position=tp)
                      else:
                          nc.tensor.matmul(pi[:], wih[:, 2 * GH : 3 * GH], xc,
                                           start=False, stop=True)
                      n_c = wk.tile([GH, NC], F16, tag="n")
                      if zb:
                          nc.scalar.activation(n_c[:], pi[:], AF.Tanh)
                      else:
                          nc.scalar.activation(n_c[:], pi[:], AF.Tanh,
                                               bias=gb[(layer, "in")][:])
                      # h' = n + z*(h - n), all DVE at fp16 2x
                      d_c = wk.tile([GH, NC], F16, tag="d_")
                      nc.vector.tensor_sub(d_c[:], hc, n_c[:])
                      e_c = wk.tile([GH, NC], F16, tag="e_")
                      nc.vector.tensor_mul(e_c[:], rz[:, NC : 2 * NC], d_c[:])
                      nc.vector.tensor_add(hc, n_c[:], e_c[:])

              for t in range(MN):
                  if t % 4 == 0:
                      emitA(t // 4, lowp=(t > 0))
                  emitB_layer(t, 0, h1, wih0, whh0, t % 4)
                  emitB_layer(t, 1, h2, wih1, whh1, None)

              # ---- phase C: route mean + node MLP + masked softmax ----------
              mean32 = fp_.tile([GH, BLOC], F32, tag="mean32")
              h2v = h2[:].rearrange("p (b r) -> p b r", r=MR)
              nc.vector.tensor_reduce(mean32[:], h2v, mybir.AxisListType.X, OP.add)
              mean = fp_.tile([GH, BLOC], F16, tag="mean")
              nc.vector.tensor_copy(mean[:], mean32[:])
              pmt = ps1.tile([BLOC, 256], F32, tag="ph", name="cpmt")
              nc.tensor.matmul(pmt[:], mean[:], wn1b[:])
              mmt = fp_.tile([BLOC, 256], F16, tag="mmt")
              nc.vector.tensor_copy(mmt[:], pmt[:])

              for c in range(NCH):
                  c0, c1 = c * NC, (c + 1) * NC
                  n1 = []
                  for m in range(2):
                      p1 = ps2.tile([128, NC], F32, tag="przb", name=f"cp1_{c}_{m}")
                      nc.tensor.matmul(
                          p1[:], wn1a[:, 128 * m : 128 * (m + 1)], h2[:, c0:c1],
                          start=True, stop=False,
                      )
                      nc.tensor.matmul(
                          p1[:], mmt[:, 128 * m : 128 * (m + 1)], sel[:, c0:c1],
                          start=False, stop=True,
                      )
                      a1 = fp_.tile([128, NC], F16, tag=f"n1_{m}")
                      if zb:
                          nc.scalar.activation(a1[:], p1[:], AF.Relu)
                      else:
                          nc.scalar.activation(a1[:], p1[:], AF.Relu,
                                               bias=bn1[:, m : m + 1])
                      n1.append(a1)
                  p2 = ps1.tile([128, NC], F32, tag="ph", name=f"cp2_{c}")
                  nc.tensor.matmul(p2[:], wn2a[:], n1[0][:], start=True, stop=False)
                  nc.tensor.matmul(p2[:], wn2b[:], n1[1][:], start=False, stop=True)
                  n2 = fp_.tile([128, NC], F16, tag="n2")
                  if zb:
                      nc.scalar.activation(n2[:], p2[:], AF.Relu)
                  else:
                      nc.scalar.activation(n2[:], p2[:], AF.Relu, bias=bn2[:])
                  for q in range(NC // 128):
                      tok0 = c0 + q * 128
                      pl = ps1.tile([128, MN], F32, tag="pi", name=f"cpl_{c}_{q}")
                      if zb:
                          nc.tensor.matmul(pl[:], n2[:, q * 128 : (q + 1) * 128], wn3[:])
                      else:
                          nc.tensor.matmul(
                              pl[:], n2[:, q * 128 : (q + 1) * 128], wn3[:],
                              start=True, stop=False,
                          )
                          nc.tensor.matmul(pl[:], ones128[:], bn3[:],
                                           start=False, stop=True)
                      ex = fp_.tile([128, MN], F32, tag="ex")
                      sm = fp_.tile([128, 1], F32, tag="sm")
                      nc.scalar.activation(ex[:], pl[:], AF.Exp, accum_out=sm[:])
                      rec = fp_.tile([128, 1], F32, tag="rec")
                      nc.vector.reciprocal(rec[:], sm[:])
                      msk = fp_.tile([128, MN], F32, tag="msk")
                      nc.vector.tensor_scalar(
                          msk[:], iota24[:], rn_pm[:, tok0 // 128 : tok0 // 128 + 1],
                          None, OP.is_lt
                      )
                      po = fp_.tile([128, MN], F32, tag="po")
                      nc.vector.scalar_tensor_tensor(
                          po[:], ex[:], rec[:], msk[:], OP.mult, OP.mult
                      )
                      nc.sync.dma_start(d_out.ap()[tok0 : tok0 + 128, :], po[:])

    nc.compile()
    return nc


def _prep_inputs(inputs, zb):
    """Host-side preprocessing -> list of per-core input dicts."""
    state = np.ascontiguousarray(inputs["state"], dtype=np.float32)
    rn = state[:, :MR]                                    # [B, 48]
    cust = state[:, MR:].reshape(B, MR, MN, FEAT)

    def f32(x):
        return np.ascontiguousarray(np.asarray(x, dtype=np.float32))

    Wih0 = f32(inputs["Wih0"]); Whh0 = f32(inputs["Whh0"])
    Wih1 = f32(inputs["Wih1"]); Whh1 = f32(inputs["Whh1"])

    com = {
        "Wc1h": np.ascontiguousarray(np.asarray(inputs["Wc1"], np.float16)),
        "Wc2h": np.ascontiguousarray(np.asarray(inputs["Wc2"], np.float16)),
        "Wih0h": np.ascontiguousarray(np.tile(np.asarray(Wih0, np.float16), (4, 1))),
        "Whh0h": Whh0.astype(np.float16),
        "Wih1h": Wih1.astype(np.float16),
        "Whh1h": Whh1.astype(np.float16),
        "Wn1a": f32(inputs["Wn1"])[0:GH, :].astype(np.float16),
        "Wn1b": (f32(inputs["Wn1"])[GH:, :] / np.float32(MR)).astype(np.float16),
        "Wn2a": f32(inputs["Wn2"])[0:128, :].astype(np.float16),
        "Wn2b": f32(inputs["Wn2"])[128:256, :].astype(np.float16),
        "Wn3h": np.asarray(inputs["Wn3"], np.float16),
        "iota24": np.tile(np.arange(MN, dtype=np.float32), (128, 1)),
        "ident128": np.eye(128, dtype=np.float16),
    }
    if not zb:
        bih0 = f32(inputs["bih0"]); bhh0 = f32(inputs["bhh0"])
        bih1 = f32(inputs["bih1"]); bhh1 = f32(inputs["bhh1"])
        com.update({
            "bc1": f32(inputs["bc1"]).reshape(CH, 1),
            "bc2s": np.tile(f32(inputs["bc2"]).reshape(CO), 4).reshape(128, 1),
            "b0_r": (bih0[0:GH] + bhh0[0:GH]).reshape(GH, 1),
            "b0_z": (bih0[GH : 2 * GH] + bhh0[GH : 2 * GH]).reshape(GH, 1),
            "b0_in": bih0[2 * GH :].reshape(GH, 1),
            "b0_hn": bhh0[2 * GH :].reshape(GH, 1),
            "b1_r": (bih1[0:GH] + bhh1[0:GH]).reshape(GH, 1),
            "b1_z": (bih1[GH : 2 * GH] + bhh1[GH : 2 * GH]).reshape(GH, 1),
            "b1_in": bih1[2 * GH :].reshape(GH, 1),
            "b1_hn": bhh1[2 * GH :].reshape(GH, 1),
            "bn1c": np.ascontiguousarray(f32(inputs["bn1"]).reshape(2, 128).T),
            "bn2c": f32(inputs["bn2"]).reshape(128, 1),
            "bn3r": f32(inputs["bn3"]).reshape(1, MN),
            "ones128": np.ones((1, 128), np.float32),
        })
    sel = np.zeros((BLOC, S), np.float32)
    sel[np.arange(S) // MR, np.arange(S)] = 1.0
    com["sel"] = sel.astype(np.float16)

    in_maps = []
    for core in range(NCORES):
        b0, b1 = core * BLOC, (core + 1) * BLOC
        # cust_fm[f, n*S + (b*MR+r)] = cust[b, r, n, f]
        cfm = cust[b0:b1].transpose(3, 2, 0, 1).reshape(FEAT, MN * S)
        m = dict(com)
        m["cust_fm"] = np.ascontiguousarray(cfm.astype(np.float16))
        # rn_pm[p, q] = route_num of token q*128+p
        m["rn_pm"] = np.ascontiguousarray(
            rn[b0:b1].reshape(S).reshape(S // 128, 128).T
        )
        in_maps.append(m)
    return in_maps


def _zb(inputs):
    return all(
        float(np.abs(np.asarray(inputs[k], np.float32)).max()) == 0.0
        for k in ("bc1", "bc2", "bih0", "bhh0", "bih1", "bhh1",
                  "bn1", "bn2", "bn3")
    )


def _run(inputs, **kw):
    zb = _zb(inputs)
    key = ("nc", zb)
    if key not in _cache:
        _cache[key] = _build(zb=zb)
    nc = _cache[key]
    _cache["nc"] = nc  # for test harness introspection
    in_maps = _prep_inputs(inputs, zb)
    return run_bass_kernel_spmd(nc, in_maps, core_ids=list(range(NCORES)), **kw)


def kernel(**inputs) -> np.ndarray:
    res = _run(inputs)
    outs = [r["out_tm"] for r in res.results]
    return np.concatenate(outs, axis=0).reshape(B, MR, MN)


# revision 20
# speedup vs baseline: 1.3095x; 1.0465x over previous
"""Trainium2 Bass kernel for nn_MLP_Route_RL_Model (route RL model).

Reference math (per batch element b of 256):
  - state = [route_nums (48) | customers (48*24*36)]
  - customer MLP (tanh-tanh, 36->128->32) on every node of every route
  - 2-layer GRU (hidden 128) over the 24 nodes of each of the 48 routes
  - route summary mean, node-selection MLP 256->256->128->24, masked softmax

Sharding: pure data parallel over batch B=256 -> 8 cores x 32.

Layout: feature-major activations ([feature, token] in SBUF) so matmuls
contract over the partition dim without transposes.

v4 schedule notes (the kernel is ACT elementwise-bound; engine cost is
free-dim elements only, so fewer/bigger ACT ops win):
  - r|z gate pre-activations share one 2-bank PSUM tile [128,1024]; with
    the (all-zero) biases dropped, ONE sigmoid covers both gates.
  - The n-gate input for all three 512-token chunks accumulates into one
    3-bank PSUM tile [128,1536]: x-side matmul + an identity matmul that
    injects t_ = r*ph (computed on DVE) into the bank; ONE tanh covers
    the full token width. This also deletes the s_ = pi + t_ DVE add.
  - h' = n + z*(h-n): 3 DVE ops/chunk at fp16 2x.
  - PSUM: przb (2 banks x2) + pi3 (3 banks x1) + ph (1) = 8 banks; the
    customer MLP borrows przb slots at low scheduler priority.
  - A non-zero-bias fallback keeps split sigmoids + bias ports.
"""

import os
import sys

import numpy as np

sys.path.insert(0, "/opt/trn_rl_repo")

import concourse.bass as bass  # noqa: E402
import concourse.bacc as bacc  # noqa: E402
import concourse.mybir as mybir  # noqa: E402
import concourse.tile as tile  # noqa: E402
from concourse.bass_utils import run_bass_kernel_spmd  # noqa: E402

F32 = mybir.dt.float32
F16 = mybir.dt.float16
AF = mybir.ActivationFunctionType
OP = mybir.AluOpType

# Problem shape constants
B = 256
NCORES = 8
BLOC = B // NCORES          # 32 batch rows per core
MR = 48                     # routes per batch
MN = 24                     # nodes per route
FEAT = 36
CH = 128                    # customer hidden
CO = 32                     # customer out
GH = 128                    # GRU hidden
S = BLOC * MR               # sequences per core = 1536
NC = 512                    # token chunk (PSUM bank width in fp32)
NCH = S // NC               # chunks per core = 3
NG = MN // 4                # node groups of 4 (cust_out partition stacking)

_cache = {}


def _build(reps=1, zb=True):
    """Trace + schedule the per-core Tile kernel. zb: all biases are zero."""
    nc = bacc.Bacc("TRN2", target_bir_lowering=False, debug=False)

    # ---- DRAM I/O ----------------------------------------------------------
    d_cust = nc.dram_tensor("cust_fm", [FEAT, MN * S], F16, kind="ExternalInput")
    d_rn = nc.dram_tensor("rn_pm", [128, S // 128], F32, kind="ExternalInput")
    d_wc1 = nc.dram_tensor("Wc1h", [FEAT, CH], F16, kind="ExternalInput")
    d_wc2 = nc.dram_tensor("Wc2h", [CH, CO], F16, kind="ExternalInput")
    d_wih0 = nc.dram_tensor("Wih0h", [128, 3 * GH], F16, kind="ExternalInput")
    d_whh0 = nc.dram_tensor("Whh0h", [GH, 3 * GH], F16, kind="ExternalInput")
    d_wih1 = nc.dram_tensor("Wih1h", [GH, 3 * GH], F16, kind="ExternalInput")
    d_whh1 = nc.dram_tensor("Whh1h", [GH, 3 * GH], F16, kind="ExternalInput")
    d_ident = nc.dram_tensor("ident128", [128, 128], F16, kind="ExternalInput")
    if not zb:
        d_bc1 = nc.dram_tensor("bc1", [CH, 1], F32, kind="ExternalInput")
        d_bc2 = nc.dram_tensor("bc2s", [128, 1], F32, kind="ExternalInput")
        d_gb = {}
        for layer in (0, 1):
            for g in ("r", "z", "in", "hn"):
                d_gb[(layer, g)] = nc.dram_tensor(
                    f"b{layer}_{g}", [GH, 1], F32, kind="ExternalInput"
                )
        d_bn1 = nc.dram_tensor("bn1c", [128, 2], F32, kind="ExternalInput")
        d_bn2 = nc.dram_tensor("bn2c", [128, 1], F32, kind="ExternalInput")
        d_bn3 = nc.dram_tensor("bn3r", [1, MN], F32, kind="ExternalInput")
        d_ones = nc.dram_tensor("ones128", [1, 128], F32, kind="ExternalInput")
    d_wn1a = nc.dram_tensor("Wn1a", [GH, 256], F16, kind="ExternalInput")
    d_wn1b = nc.dram_tensor("Wn1b", [GH, 256], F16, kind="ExternalInput")
    d_wn2a = nc.dram_tensor("Wn2a", [128, 128], F16, kind="ExternalInput")
    d_wn2b = nc.dram_tensor("Wn2b", [128, 128], F16, kind="ExternalInput")
    d_wn3 = nc.dram_tensor("Wn3h", [GH, MN], F16, kind="ExternalInput")
    d_sel = nc.dram_tensor("sel", [BLOC, S], F16, kind="ExternalInput")
    d_iota = nc.dram_tensor("iota24", [128, MN], F32, kind="ExternalInput")
    d_out = nc.dram_tensor("out_tm", [S, MN], F32, kind="ExternalOutput")

    with tile.TileContext(nc) as tc:
        with (
            tc.tile_pool(name="wpool", bufs=1) as wp,
            tc.tile_pool(name="state", bufs=1) as sp,
            tc.tile_pool(name="xin", bufs=10) as xp,
            tc.tile_pool(name="h1c", bufs=6) as h1p,
            tc.tile_pool(name="wk", bufs=6) as wk,
            tc.tile_pool(name="nfw", bufs=3) as nfw,
            tc.tile_pool(name="fin", bufs=4) as fp_,
            tc.tile_pool(name="ps2", bufs=2, space="PSUM") as ps2,
            tc.tile_pool(name="ps1", bufs=2, space="PSUM") as ps1,
        ):
            def lowprio():
                # deprioritize: scheduler runs these only in recurrence gaps
                return tc.high_priority(offset=-1_000_000)

            # ---- load weights / constants (A-critical ones first) ----------
            def wtile(dram, shape, dtype):
                t = wp.tile(shape, dtype, tag=dram.name)
                nc.sync.dma_start(t[:], dram.ap())
                return t

            wc1 = wtile(d_wc1, [FEAT, CH], F16)
            wc2 = wtile(d_wc2, [CH, CO], F16)
            ident = wtile(d_ident, [128, 128], F16)
            wih0 = wtile(d_wih0, [128, 3 * GH], F16)
            whh0 = wtile(d_whh0, [GH, 3 * GH], F16)
            wih1 = wtile(d_wih1, [128, 3 * GH], F16)
            whh1 = wtile(d_whh1, [GH, 3 * GH], F16)
            gb = {}
            if not zb:
                bc1 = wtile(d_bc1, [CH, 1], F32)
                bc2 = wtile(d_bc2, [128, 1], F32)
                for k, d in d_gb.items():
                    gb[k] = wtile(d, [GH, 1], F32)
            with lowprio():
                wn1a = wtile(d_wn1a, [GH, 256], F16)
                wn1b = wtile(d_wn1b, [GH, 256], F16)
                wn2a = wtile(d_wn2a, [128, 128], F16)
                wn2b = wtile(d_wn2b, [128, 128], F16)
                wn3 = wtile(d_wn3, [GH, MN], F16)
                sel = wtile(d_sel, [BLOC, S], F16)
                iota24 = wtile(d_iota, [128, MN], F32)
                rn_pm = wtile(d_rn, [128, S // 128], F32)
                if not zb:
                    bn1 = wtile(d_bn1, [128, 2], F32)
                    bn2 = wtile(d_bn2, [128, 1], F32)
                    bn3 = wtile(d_bn3, [1, MN], F32)
                    ones128 = wtile(d_ones, [1, 128], F32)

            # persistent state: customer-MLP output, GRU hidden states
            # cust_out layout: partition = (n%4)*32 + f, free = (n//4)*S + s
            cust = sp.tile([128, NG * S], F16, tag="cust_out")
            h1 = sp.tile([GH, S], F16, tag="h1")
            h2 = sp.tile([GH, S], F16, tag="h2")

          # timing-calibration repeat loop (reps=1 in production)
          # fmt: off
            for _rep in range(reps):
              nc.vector.memset(h1[:], 0.0)
              nc.gpsimd.memset(h2[:], 0.0)

              # ---- phase A: customer MLP (gap filler) ----------------------
              # p1 pre-activations for node pairs share a przb 2-bank tile;
              # one tanh covers both (bc1 is per-partition so this also
              # holds in the non-zb fallback).
              xtiles = {}
              import contextlib
              def emitA(g, lowp):
                with (lowprio() if lowp else contextlib.nullcontext()):
                  for sb in range(NCH):
                      # stage 1: h1c for the 4 nodes (2 przb borrows, one at
                      # a time); stage 2: c2 accumulation (1 przb borrow).
                      h1cbs = []
                      for kp in range(2):
                          p1b = ps2.tile([CH, 2 * NC], F32, tag="przb",
                                         name=f"p1b_{g}_{sb}_{kp}")
                          h1cb = h1p.tile([CH, 2 * NC], F16, tag="h1c")
                          for kk in range(2):
                              k = 2 * kp + kk
                              n = 4 * g + k
                              if n not in xtiles:
                                  xn = xp.tile([FEAT, S], F16, tag="xc", name=f"xc{n}")
                                  nc.sync.dma_start(
                                      xn[:], d_cust.ap()[:, n * S : (n + 1) * S]
                                  )
                                  xtiles[n] = xn
                              xc = xtiles[n]
                              nc.tensor.matmul(
                                  p1b[:, kk * NC : (kk + 1) * NC], wc1[:],
                                  xc[:, sb * NC : (sb + 1) * NC],
                              )
                          if zb:
                              nc.scalar.activation(h1cb[:], p1b[:], AF.Tanh)
                          else:
                              nc.scalar.activation(h1cb[:], p1b[:], AF.Tanh, bias=bc1[:])
                          h1cbs.append(h1cb)
                      c2 = ps1.tile([128, NC], F32, tag="pi", name=f"c2_{g}_{sb}")
                      for k in range(4):
                          nc.tensor.matmul(
                              c2[32 * k : 32 * (k + 1), :], wc2[:],
                              h1cbs[k // 2][:, (k % 2) * NC : (k % 2 + 1) * NC],
                              tile_position=(0, 32 * k),
                          )
                      if zb:
                          nc.scalar.activation(
                              cust[:, g * S + sb * NC : g * S + (sb + 1) * NC],
                              c2[:], AF.Tanh,
                          )
                      else:
                          nc.scalar.activation(
                              cust[:, g * S + sb * NC : g * S + (sb + 1) * NC],
                              c2[:], AF.Tanh, bias=bc2[:],
                          )

              # ---- phase B: 2-layer GRU over MN steps -----------------------
              def emitB_layer(t, layer, h, wih, whh, kq):
                  """One GRU layer update for step t on hidden h [GH, S]."""
                  g = t // 4
                  for c in range(NCH):
                      c0, c1 = c * NC, (c + 1) * NC
                      hc = h[:, c0:c1]
                      przb = ps2.tile([GH, 2 * NC], F32, tag="przb")
                      pr = przb[:, 0:NC]
                      pz = przb[:, NC : 2 * NC]
                      ph = ps1.tile([GH, NC], F32, tag="ph")
                      pi = ps1.tile([GH, NC], F32, tag="pi")
                      if kq is not None:
                          p0 = 32 * kq
                          tp = (p0, 0)
                          xc = cust[p0 : p0 + CO, g * S + c0 : g * S + c1]
                          wx = wih[p0 : p0 + CO, :]
                          nc.tensor.matmul(pr, whh[:, 0:GH], hc, start=True, stop=False)
                          nc.tensor.matmul(pr, wx[:, 0:GH], xc, start=False, stop=True,
                                           tile_position=tp)
                          nc.tensor.matmul(pz, whh[:, GH : 2 * GH], hc, start=True, stop=False)
                          nc.tensor.matmul(pz, wx[:, GH : 2 * GH], xc, start=False, stop=True,
                                           tile_position=tp)
                          nc.tensor.matmul(ph[:], whh[:, 2 * GH : 3 * GH], hc)
                      else:
                          xc = h1[:, c0:c1]
                          nc.tensor.matmul(pr, whh[:, 0:GH], hc, start=True, stop=False)
                          nc.tensor.matmul(pr, wih[:, 0:GH], xc, start=False, stop=True)
                          nc.tensor.matmul(pz, whh[:, GH : 2 * GH], hc, start=True, stop=False)
                          nc.tensor.matmul(pz, wih[:, GH : 2 * GH], xc, start=False, stop=True)
                          nc.tensor.matmul(ph[:], whh[:, 2 * GH : 3 * GH], hc)
                      rz = wk.tile([GH, 2 * NC], F16, tag="rz")
                      if zb:
                          nc.scalar.activation(rz[:], przb[:], AF.Sigmoid)
                      else:
                          nc.scalar.activation(rz[:, 0:NC], pr, AF.Sigmoid,
                                               bias=gb[(layer, "r")][:])
                          nc.scalar.activation(rz[:, NC : 2 * NC], pz, AF.Sigmoid,
                                               bias=gb[(layer, "z")][:])
                      t_c = wk.tile([GH, NC], F16, tag="t_")
                      if zb:
                          nc.vector.tensor_mul(t_c[:], ph[:], rz[:, 0:NC])
                      else:
                          nc.vector.scalar_tensor_tensor(
                              t_c[:], ph[:], gb[(layer, "hn")][:], rz[:, 0:NC],
                              OP.add, OP.mult,
                          )
                      # finish the n-gate input inside PSUM: pi = I@t_ + Wih_n@x
                      nc.tensor.matmul(pi[:], ident[:], t_c[:], start=True, stop=False)
                      if kq is not None:
                          nc.tensor.matmul(pi[:], wx[:, 2 * GH : 3 * GH], xc,
                                           start=False, stop=True, tile_position=tp)
                      else:
                          nc.tensor.matmul(pi[:], wih[:, 2 * GH : 3 * GH], xc,
                                           start=False, stop=True)
                      n_c = wk.tile([GH, NC], F16, tag="n")
                      if zb:
                          nc.scalar.activation(n_c[:], pi[:], AF.Tanh)
                      else:
                          nc.scalar.activation(n_c[:], pi[:], AF.Tanh,
                                               bias=gb[(layer, "in")][:])
                      # h' = n + z*(h - n), all DVE at fp16 2x
                      d_c = wk.tile([GH, NC], F16, tag="d_")
                      nc.vector.tensor_sub(d_c[:], hc, n_c[:])
                      e_c = wk.tile([GH, NC], F16, tag="e_")
                      nc.vector.tensor_mul(e_c[:], rz[:, NC : 2 * NC], d_c[:])
                      nc.vector.tensor_add(hc, n_c[:], e_c[:])

              # customer MLP runs one 4-node group AHEAD of the GRU so a
              # group boundary never stalls L1 on a just-in-time cust tile.
              emitA(0, lowp=False)
              for t in range(MN):
                  if t % 4 == 0 and t // 4 + 1 < NG:
                      emitA(t // 4 + 1, lowp=True)
                  emitB_layer(t, 0, h1, wih0, whh0, t % 4)
                  emitB_layer(t, 1, h2, wih1, whh1, None)

              # ---- phase C: route mean + node MLP + masked softmax ----------
              mean32 = fp_.tile([GH, BLOC], F32, tag="mean32")
              h2v = h2[:].rearrange("p (b r) -> p b r", r=MR)
              nc.vector.tensor_reduce(mean32[:], h2v, mybir.AxisListType.X, OP.add)
              mean = fp_.tile([GH, BLOC], F16, tag="mean")
              nc.vector.tensor_copy(mean[:], mean32[:])
              pmt = ps1.tile([BLOC, 256], F32, tag="ph", name="cpmt")
              nc.tensor.matmul(pmt[:], mean[:], wn1b[:])
              mmt = fp_.tile([BLOC, 256], F16, tag="mmt")
              nc.vector.tensor_copy(mmt[:], pmt[:])

              for c in range(NCH):
                  c0, c1 = c * NC, (c + 1) * NC
                  n1 = []
                  for m in range(2):
                      p1 = ps2.tile([128, NC], F32, tag="przb", name=f"cp1_{c}_{m}")
                      nc.tensor.matmul(
                          p1[:], wn1a[:, 128 * m : 128 * (m + 1)], h2[:, c0:c1],
                          start=True, stop=False,
                      )
                      nc.tensor.matmul(
                          p1[:], mmt[:, 128 * m : 128 * (m + 1)], sel[:, c0:c1],
                          start=False, stop=True,
                      )
                      a1 = fp_.tile([128, NC], F16, tag=f"n1_{m}")
                      if zb:
                          nc.scalar.activation(a1[:], p1[:], AF.Relu)
                      else:
                          nc.scalar.activation(a1[:], p1[:], AF.Relu,
                                               bias=bn1[:, m : m + 1])
                      n1.append(a1)
                  p2 = ps1.tile([128, NC], F32, tag="ph", name=f"cp2_{c}")
                  nc.tensor.matmul(p2[:], wn2a[:], n1[0][:], start=True, stop=False)
                  nc.tensor.matmul(p2[:], wn2b[:], n1[1][:], start=False, stop=True)
                  n2 = fp_.tile([128, NC], F16, tag="n2")
                  if zb:
                      nc.scalar.activation(n2[:], p2[:], AF.Relu)
                  else:
                      nc.scalar.activation(n2[:], p2[:], AF.Relu, bias=bn2[:])
                  for q in range(NC // 128):
                      tok0 = c0 + q * 128
                      pl = ps1.tile([128, MN], F32, tag="pi", name=f"cpl_{c}_{q}")
                      if zb:
                          nc.tensor.matmul(pl[:], n2[:, q * 128 : (q + 1) * 128], wn3[:])
                      else:
                          nc.tensor.matmul(
                              pl[:], n2[:, q * 128 : (q + 1) * 128], wn3[:],
                              start=True, stop=False,
                          )
                          nc.tensor.matmul(pl[:], ones128[:], bn3[:],
                                           start=False, stop=True)
                      ex = fp_.tile([128, MN], F32, tag="ex")
                      sm = fp_.tile([128, 1], F32, tag="sm")
                      nc.scalar.activation(ex[:], pl[:], AF.Exp, accum_out=sm[:])
                      rec = fp_.tile([128, 1], F32, tag="rec")
                      nc.vector.reciprocal(rec[:], sm[:])
                      msk = fp_.tile([128, MN], F32, tag="msk")
                      nc.vector.tensor_scalar(
                          msk[:], iota24[:], rn_pm[:, tok0 // 128 : tok0 // 128 + 1],
                          None, OP.is_lt
                      )
                      po = fp_.tile([128, MN], F32, tag="po")
                      nc.vector.scalar_tensor_tensor(
                          po[:], ex[:], rec[:], msk[:], OP.mult, OP.mult
                      )
                      nc.sync.dma_start(d_out.ap()[tok0 : tok0 + 128, :], po[:])

    nc.compile()
    return nc


def _prep_inputs(inputs, zb):
    """Host-side preprocessing -> list of per-core input dicts."""
    state = np.ascontiguousarray(inputs["state"], dtype=np.float32)
    rn = state[:, :MR]                                    # [B, 48]
    cust = state[:, MR:].reshape(B, MR, MN, FEAT)

    def f32(x):
        return np.ascontiguousarray(np.asarray(x, dtype=np.float32))

    Wih0 = f32(inputs["Wih0"]); Whh0 = f32(inputs["Whh0"])
    Wih1 = f32(inputs["Wih1"]); Whh1 = f32(inputs["Whh1"])

    com = {
        "Wc1h": np.ascontiguousarray(np.asarray(inputs["Wc1"], np.float16)),
        "Wc2h": np.ascontiguousarray(np.asarray(inputs["Wc2"], np.float16)),
        "Wih0h": np.ascontiguousarray(np.tile(np.asarray(Wih0, np.float16), (4, 1))),
        "Whh0h": Whh0.astype(np.float16),
        "Wih1h": Wih1.astype(np.float16),
        "Whh1h": Whh1.astype(np.float16),
        "Wn1a": f32(inputs["Wn1"])[0:GH, :].astype(np.float16),
        "Wn1b": (f32(inputs["Wn1"])[GH:, :] / np.float32(MR)).astype(np.float16),
        "Wn2a": f32(inputs["Wn2"])[0:128, :].astype(np.float16),
        "Wn2b": f32(inputs["Wn2"])[128:256, :].astype(np.float16),
        "Wn3h": np.asarray(inputs["Wn3"], np.float16),
        "iota24": np.tile(np.arange(MN, dtype=np.float32), (128, 1)),
        "ident128": np.eye(128, dtype=np.float16),
    }
    if not zb:
        bih0 = f32(inputs["bih0"]); bhh0 = f32(inputs["bhh0"])
        bih1 = f32(inputs["bih1"]); bhh1 = f32(inputs["bhh1"])
        com.update({
            "bc1": f32(inputs["bc1"]).reshape(CH, 1),
            "bc2s": np.tile(f32(inputs["bc2"]).reshape(CO), 4).reshape(128, 1),
            "b0_r": (bih0[0:GH] + bhh0[0:GH]).reshape(GH, 1),
            "b0_z": (bih0[GH : 2 * GH] + bhh0[GH : 2 * GH]).reshape(GH, 1),
            "b0_in": bih0[2 * GH :].reshape(GH, 1),
            "b0_hn": bhh0[2 * GH :].reshape(GH, 1),
            "b1_r": (bih1[0:GH] + bhh1[0:GH]).reshape(GH, 1),
            "b1_z": (bih1[GH : 2 * GH] + bhh1[GH : 2 * GH]).reshape(GH, 1),
            "b1_in": bih1[2 * GH :].reshape(GH, 1),
            "b1_hn": bhh1[2 * GH :].reshape(GH, 1),
            "bn1c": np.ascontiguousarray(f32(inputs["bn1"]).reshape(2, 128).T),
            "bn2c": f32(inputs["bn2"]).reshape(128, 1),
            "bn3r": f32(inputs["bn3"]).reshape(1, MN),
            "ones128": np.ones((1, 128), np.float32),
        })
    sel = np.zeros((BLOC, S), np.float32)
    sel[np.arange(S) // MR, np.arange(S)] = 1.0
    com["sel"] = sel.astype(np.float16)

    in_maps = []
    for core in range(NCORES):
        b0, b1 = core * BLOC, (core + 1) * BLOC
        # cust_fm[f, n*S + (b*MR+r)] = cust[b, r, n, f]
        cfm = cust[b0:b1].transpose(3, 2, 0, 1).reshape(FEAT, MN * S)
        m = dict(com)
        m["cust_fm"] = np.ascontiguousarray(cfm.astype(np.float16))
        # rn_pm[p, q] = route_num of token q*128+p
        m["rn_pm"] = np.ascontiguousarray(
            rn[b0:b1].reshape(S).reshape(S // 128, 128).T
        )
        in_maps.append(m)
    return in_maps


def _zb(inputs):
    return all(
        float(np.abs(np.asarray(inputs[k], np.float32)).max()) == 0.0
        for k in ("bc1", "bc2", "bih0", "bhh0", "bih1", "bhh1",
                  "bn1", "bn2", "bn3")
    )


def _run(inputs, **kw):
    zb = _zb(inputs)
    key = ("nc", zb)
    if key not in _cache:
        _cache[key] = _build(zb=zb)
    nc = _cache[key]
    _cache["nc"] = nc  # for test harness introspection
    in_maps = _prep_inputs(inputs, zb)
    return run_bass_kernel_spmd(nc, in_maps, core_ids=list(range(NCORES)), **kw)


def kernel(**inputs) -> np.ndarray:
    res = _run(inputs)
    outs = [r["out_tm"] for r in res.results]
    return np.concatenate(outs, axis=0).reshape(B, MR, MN)
